# revision 1
# baseline (speedup 1.0000x reference)
"""Bass/Trainium2 kernel for nn_BoxFilter: 9x9 circular box-mean over
(8, 3, 1024, 1024) f32, data-parallel across 8 NeuronCores (1 image/core).

Pipeline per core, per channel, in blocks of 120 output rows:
  - input arrives as bf16 hi/lo pairs (packed host-side during sharding;
    same 4 B/pixel DMA volume as fp32, fp32-accurate after PSUM accumulate)
  - vertical pass: banded ones-matmuls on PE (hi + lo accumulate in PSUM)
  - 1/81 scaling folded into the ACT PSUM->SBUF copy
  - horizontal pass: one DVE tensor_tensor_scan running-box recurrence
    state[t] = state[t-1] + u[t] - u[t-9] over a wrap-padded row buffer
  - loads issue on the Sync HWDGE ring, stores on the Scalar ring, with
    blocks paired into ~1 MB transfers.
"""

import numpy as np
import ml_dtypes

import concourse.bacc as bacc
import concourse.mybir as mybir
import concourse.tile as tile
from concourse.ap import AP
from concourse.bass_utils import run_bass_kernel_spmd

B, C, H, W = 8, 3, 1024, 1024
R = 4            # filter radius
WIN = 2 * R + 1  # 9
AREA = WIN * WIN
MBLK = 120       # output rows per block (input rows = MBLK + 2R = 128)
NBLK = (H + MBLK - 1) // MBLK  # 9 (last block has 64 rows)
UW = WIN + W + 2 * R  # u buffer: [9 zeros | left wrap 4 | row 1024 | right wrap 4]

_CACHE: dict = {}


def _band_weights() -> np.ndarray:
    w = np.zeros((128, MBLK), dtype=ml_dtypes.bfloat16)
    for m in range(MBLK):
        w[m : m + WIN, m] = 1.0
    return w


def _pack_image(x: np.ndarray) -> np.ndarray:
    """[C,H,W] f32 -> [C,H,2,W] bf16 (hi, lo) with hi+lo ~= x."""
    hi = x.astype(ml_dtypes.bfloat16)
    lo = (x - hi.astype(np.float32)).astype(ml_dtypes.bfloat16)
    return np.ascontiguousarray(np.stack([hi, lo], axis=2))


def _build():
    f32 = mybir.dt.float32
    bf16 = mybir.dt.bfloat16
    nc = bacc.Bacc("TRN2", target_bir_lowering=False, debug=False, num_devices=B)
    x_d = nc.dram_tensor("x", [C, H, 2, W], bf16, kind="ExternalInput")
    w_d = nc.dram_tensor("w", [128, MBLK], bf16, kind="ExternalInput")
    o_d = nc.dram_tensor("o", [C, H, W], f32, kind="ExternalOutput")
    XROW = 2 * W              # one packed image row (bf16 elements)
    XCH = H * XROW

    def vertical(v_t, x_t, w_t, m, k, q):
        for n in range(0, W, 512):
            for s in range(2):
                nc.tensor.matmul(
                    v_t[0:m, n : n + 512],
                    w_t[0:k, 0:m],
                    x_t[0:k, q, s * W + n : s * W + n + 512],
                    start=(s == 0),
                    stop=(s == 1),
                )

    def horizontal(o_t, v_t, u_t, m, oq):
        """u = [zeros(9) | v[1020:]/81 | v/81 | v[:4]/81]; one DVE box scan."""
        nc.vector.memset(u_t[0:m, 0:WIN], 0.0)
        nc.scalar.mul(out=u_t[0:m, WIN : WIN + R], in_=v_t[0:m, W - R : W], mul=1.0 / AREA)
        nc.scalar.mul(out=u_t[0:m, WIN + R + W : UW], in_=v_t[0:m, 0:R], mul=1.0 / AREA)
        nc.scalar.mul(out=u_t[0:m, WIN + R : WIN + R + W], in_=v_t[0:m, :], mul=1.0 / AREA)
        nc.vector.tensor_tensor_scan(
            out=o_t[0:m, oq, :],
            data0=u_t[0:m, WIN:UW],
            data1=u_t[0:m, 0 : UW - WIN],
            initial=0.0,
            op0=mybir.AluOpType.add,
            op1=mybir.AluOpType.subtract,
        )

    with tile.TileContext(nc) as tc:
        with (
            tc.tile_pool(name="wpool", bufs=1) as wpool,
            tc.tile_pool(name="xpool", bufs=8) as xpool,
            tc.tile_pool(name="x8pool", bufs=2) as x8pool,
            tc.tile_pool(name="o8pool", bufs=2) as o8pool,
            tc.tile_pool(name="upool", bufs=10) as upool,
            tc.tile_pool(name="opool", bufs=7) as opool,
            tc.tile_pool(name="psum", bufs=4, space="PSUM") as psum,
        ):
            w_t = wpool.tile([128, MBLK], bf16)
            nc.sync.dma_start(w_t[:], w_d.ap())

            def do_block8(c):
                m, k = H - 8 * MBLK, H - 8 * MBLK + 2 * R
                r0 = 8 * MBLK - R
                x8_t = x8pool.tile([128, 1, 2 * W], bf16, tag="x1")
                eng8 = nc.scalar if c == 0 else nc.sync
                eng8.dma_start(x8_t[0 : H - r0, 0, :], x_d.ap()[c, r0:H, :, :])
                eng8.dma_start(
                    x8_t[H - r0 : k, 0, :], x_d.ap()[c, 0 : k - (H - r0), :, :]
                )
                o8_t = o8pool.tile([MBLK, 1, W + 2 * R], f32, tag="o1")
                v_t = psum.tile([MBLK, W], f32, tag="v")
                vertical(v_t, x8_t, w_t, m, k, 0)
                u_t = upool.tile([128, UW], f32, tag="u")
                horizontal(o8_t, v_t, u_t, m, 0)
                nc.gpsimd.dma_start(
                    o_d.ap()[c, 8 * MBLK : H, :], o8_t[0:m, 0, 2 * R : 2 * R + W]
                )

            def do_pair(c, j):
                r0 = 240 * j - R
                x_t = xpool.tile([128, 2, 2 * W], bf16, tag="x2")
                if j == 0:
                    nc.sync.dma_start(x_t[0:R, 0, :], x_d.ap()[c, H - R : H, :, :])
                    nc.sync.dma_start(x_t[R:64, 0, :], x_d.ap()[c, 0 : 64 - R, :, :])
                    nc.scalar.dma_start(
                        x_t[64:128, 0, :], x_d.ap()[c, 64 - R : 128 - R, :, :]
                    )
                    nc.sync.dma_start(
                        x_t[0:64, 1, :], x_d.ap()[c, MBLK - R : MBLK - R + 64, :, :]
                    )
                    nc.scalar.dma_start(
                        x_t[64:128, 1, :],
                        x_d.ap()[c, MBLK - R + 64 : MBLK - R + 128, :, :],
                    )
                else:
                    nc.sync.dma_start(
                        x_t[:],
                        AP(
                            x_d,
                            c * XCH + r0 * XROW,
                            [[XROW, 128], [MBLK * XROW, 2], [1, XROW]],
                        ),
                    )
                o_t = opool.tile([MBLK, 2, W + 2 * R], f32, tag="o2")
                for q in range(2):
                    v_t = psum.tile([MBLK, W], f32, tag="v")
                    vertical(v_t, x_t, w_t, MBLK, 128, q)
                    u_t = upool.tile([128, UW], f32, tag="u")
                    horizontal(o_t, v_t, u_t, MBLK, q)
                nc.scalar.dma_start(
                    o_d.ap()[c, 2 * j * MBLK : (2 * j + 1) * MBLK, :],
                    o_t[:, 0, 2 * R : 2 * R + W],
                )
                nc.gpsimd.dma_start(
                    o_d.ap()[c, (2 * j + 1) * MBLK : (2 * j + 2) * MBLK, :],
                    o_t[:, 1, 2 * R : 2 * R + W],
                )

            # round-robin channels per step: uniform load/store streaming
            for c in range(C):
                do_block8(c)
            for j in range(4):
                for c in range(C):
                    do_pair(c, j)
    nc.compile()
    return nc


def _get_nc():
    if "nc" not in _CACHE:
        _CACHE["nc"] = _build()
    return _CACHE["nc"]


def _prepare_in_maps(tensor: np.ndarray) -> list:
    x = np.asarray(tensor, dtype=np.float32)
    assert x.shape == (B, C, H, W), x.shape
    wmat = _band_weights()
    return [{"x": _pack_image(x[i]), "w": wmat} for i in range(B)]


def kernel(tensor: np.ndarray) -> np.ndarray:
    nc = _get_nc()
    in_maps = _prepare_in_maps(tensor)
    res = run_bass_kernel_spmd(nc, in_maps, core_ids=list(range(B)))
    return np.stack([res.results[i]["o"] for i in range(B)], axis=0)

